# revision 19
# baseline (speedup 1.0000x reference)
"""Multi-head attention (B=4, T=2048, C=1024, H=16, causal) on 8 TRN2 cores.

Sharding: core c -> batch b = c//2, head-half h2 = c%2 (8 heads / core).
bf16 datapath (rel tol 2e-2 >> bf16 noise). Host pre-transposes x to
[C, T] so projections need no on-chip transposes. Per 512-token window:
column-parallel K/Q projections into K^T/Q^T [feat, T] layout, V natural,
then causal attention for that query window (scores 2 heads per slot via
base-64 partition row-tiling, softmax row-sums via inline ones column in
V), normalized output written to SBUF-resident y^T, row-split output
projection, host sums the core pairs.
"""

import sys

sys.path.insert(0, "/opt/trn_rl_repo")

import numpy as np
import ml_dtypes

import concourse.bacc as bacc
import concourse.bass as bass
import concourse.mybir as mybir
import concourse.tile as tile
from concourse.bass_utils import run_bass_kernel_spmd

F32 = mybir.dt.float32
BF16 = mybir.dt.bfloat16
AF = mybir.ActivationFunctionType
BFNP = ml_dtypes.bfloat16

P = 128          # partitions
T = 2048         # sequence length
C = 1024         # model dim
FS = 512         # per-core feature slice (8 heads x 64)
NH = 8           # heads per core
HD = 64          # head dim
SCALE = 0.125    # 1/sqrt(64)
NCORES = 8

NTQ = 4          # T / 512 query windows
NFB = 4          # FS / 128 feature blocks
NCB = 8          # C / 128 contraction blocks
NTT = 16         # T / 128 key tiles


def build_program():
    nc = bacc.Bacc(num_devices=NCORES)

    xqt = nc.declare_dram_parameter("xqt", [C, T], BF16, isOutput=False)
    xkt = nc.declare_dram_parameter("xkt", [C, T], BF16, isOutput=False)
    xvt = nc.declare_dram_parameter("xvt", [C, T], BF16, isOutput=False)
    # wq/wk[p, (fb*NCB+cb)*P + j] = W[128*cb + p, 512*h2 + 128*fb + j]
    wq = nc.declare_dram_parameter("wq", [P, NFB * NCB * P], BF16, isOutput=False)
    wk = nc.declare_dram_parameter("wk", [P, NFB * NCB * P], BF16, isOutput=False)
    wv = nc.declare_dram_parameter("wv", [C, FS], BF16, isOutput=False)
    # wo[p, (cc*NFB+fc)*P + j] = Wo[512*h2 + 128*fc + p, 128*cc + j]
    wo = nc.declare_dram_parameter("wo", [P, NCB * NFB * P], BF16, isOutput=False)
    bq = nc.declare_dram_parameter("bq", [P, NFB], F32, isOutput=False)
    bk = nc.declare_dram_parameter("bk", [P, NFB], F32, isOutput=False)
    bv = nc.declare_dram_parameter("bv", [P, FS], F32, isOutput=False)
    bo = nc.declare_dram_parameter("bo", [P, NCB], F32, isOutput=False)
    # tri[p, j] = 1.0 iff j >= p  (causal mask for a 128x128 diagonal block)
    tri = nc.declare_dram_parameter("tri", [P, P], BF16, isOutput=False)
    onesp = nc.declare_dram_parameter("onesp", [P, HD], BF16, isOutput=False)
    out = nc.declare_dram_parameter("out", [C, T], BF16, isOutput=True)

    with tile.TileContext(nc) as tc:
        import contextlib

        with contextlib.ExitStack() as ctx:
            consts = ctx.enter_context(tc.tile_pool(name="consts", bufs=1))
            xw_pool = ctx.enter_context(tc.tile_pool(name="xw", bufs=2))
            kt_pool = ctx.enter_context(tc.tile_pool(name="ktp", bufs=1))
            qt_pool = ctx.enter_context(tc.tile_pool(name="qtp", bufs=1))
            v_pool = ctx.enter_context(tc.tile_pool(name="vp", bufs=1))
            ya_pool = ctx.enter_context(tc.tile_pool(name="yap", bufs=1))
            ex_pool = ctx.enter_context(tc.tile_pool(name="expp", bufs=4))
            yu_pool = ctx.enter_context(tc.tile_pool(name="yup", bufs=4))
            rc_pool = ctx.enter_context(tc.tile_pool(name="rcp", bufs=4))
            rb_pool = ctx.enter_context(tc.tile_pool(name="rbp", bufs=4))
            ob_pool = ctx.enter_context(tc.tile_pool(name="obp", bufs=3))
            # PSUM: 'ps' [P,1024]x2 = 4 banks, 'psy' x2 = 2, 'pp' x2 = 2
            psS = ctx.enter_context(tc.tile_pool(name="psS", bufs=2, space="PSUM"))
            psY = ctx.enter_context(tc.tile_pool(name="psY", bufs=2, space="PSUM"))
            psP = ctx.enter_context(tc.tile_pool(name="psP", bufs=2, space="PSUM"))
            dram = ctx.enter_context(tc.tile_pool(name="dram", bufs=4, space="DRAM"))

            # ---- constants (spread across engine DMA queues so the head
            # of the kernel isn't serialized on one queue)
            wq_sb = consts.tile([P, NFB * NCB * P], BF16, tag="wq", name="wq_sb")
            nc.sync.dma_start(wq_sb[:], wq[:])
            wk_sb = consts.tile([P, NFB * NCB * P], BF16, tag="wk", name="wk_sb")
            nc.scalar.dma_start(wk_sb[:], wk[:])
            wo_sb = consts.tile([P, NCB * NFB * P], BF16, tag="wo", name="wo_sb")
            nc.scalar.dma_start(wo_sb[:], wo[:])
            wv_sb = consts.tile([P, NCB, FS], BF16, tag="wv", name="wv_sb")
            nc.gpsimd.dma_start(
                wv_sb[:], wv[:].rearrange("(cb p) f -> p cb f", p=P)
            )
            bq_t = consts.tile([P, NFB], F32, tag="bq", name="bq_t")
            nc.sync.dma_start(bq_t[:], bq[:])
            bk_t = consts.tile([P, NFB], F32, tag="bk", name="bk_t")
            nc.sync.dma_start(bk_t[:], bk[:])
            bo_t = consts.tile([P, NCB], F32, tag="bo", name="bo_t")
            nc.sync.dma_start(bo_t[:], bo[:])
            bv_sb = consts.tile([P, FS], F32, tag="bv", name="bv_sb")
            nc.scalar.dma_start(bv_sb[:], bv[:])
            tri_sb = consts.tile([P, P], BF16, tag="tri", name="tri_sb")
            nc.scalar.dma_start(tri_sb[:], tri[:])
            ones_sb = consts.tile([P, HD], BF16, tag="ones", name="ones_sb")
            nc.gpsimd.dma_start(ones_sb[:], onesp[:])
            bq_sb = [bq_t[:, i : i + 1] for i in range(NFB)]
            bk_sb = [bk_t[:, i : i + 1] for i in range(NFB)]
            bo_sb = [bo_t[:, i : i + 1] for i in range(NCB)]

            # ---- persistent attention operands (bf16)
            KT = [kt_pool.tile([P, T], BF16, tag=f"kt{i}", name=f"kt{i}")
                  for i in range(NFB)]
            QT = [qt_pool.tile([P, T], BF16, tag=f"qt{i}", name=f"qt{i}")
                  for i in range(NFB)]
            # V tiles carry an inline ones column per head: [v_h | 1] x 8
            VSB = [v_pool.tile([P, NH * (HD + 1)], BF16, tag=f"v{i}", name=f"v{i}")
                   for i in range(NTT)]
            YA = [ya_pool.tile([P, T], BF16, tag=f"ya{i}", name=f"ya{i}")
                  for i in range(NFB)]

            for tw in range(NTQ):
                qsl = slice(512 * tw, 512 * (tw + 1))

                # ---- x^T windows: one DMA per matrix, [c, cb-major, t]
                xk_w = xw_pool.tile([P, NCB, 512], BF16, tag="xk", name="xk_w")
                nc.sync.dma_start(
                    xk_w[:], xkt[:, qsl].rearrange("(cb p) t -> p cb t", p=P)
                )
                xq_w = xw_pool.tile([P, NCB, 512], BF16, tag="xq", name="xq_w")
                nc.gpsimd.dma_start(
                    xq_w[:], xqt[:, qsl].rearrange("(cb p) t -> p cb t", p=P)
                )
                xv_w = xw_pool.tile([P, NCB, 512], BF16, tag="xv", name="xv_w")
                nc.gpsimd.dma_start(
                    xv_w[:], xvt[:, qsl].rearrange("(cb p) t -> p cb t", p=P)
                )

                # ---- K^T, Q^T projections for this window
                for wsb, xin, bias_sb, OUT in (
                    (wk_sb, xk_w, bk_sb, KT),
                    (wq_sb, xq_w, bq_sb, QT),
                ):
                    for fb in range(NFB):
                        pp = psP.tile([P, 512], F32, tag="pp", name="pp")
                        for cb in range(NCB):
                            o = (fb * NCB + cb) * P
                            nc.tensor.matmul(
                                pp[:], wsb[:, o : o + P], xin[:, cb, :],
                                start=(cb == 0), stop=(cb == NCB - 1),
                            )
                        nc.vector.tensor_scalar_add(
                            OUT[fb][:, qsl], pp[:], bias_sb[fb]
                        )

                # ---- V projection (natural layout) for this window
                for tb in range(4):
                    pv = psP.tile([P, 512], F32, tag="pp", name="pv")
                    for cb in range(NCB):
                        nc.tensor.matmul(
                            pv[:], xv_w[:, cb, 128 * tb : 128 * (tb + 1)],
                            wv_sb[:, cb, :],
                            start=(cb == 0), stop=(cb == NCB - 1),
                        )
                    vt = VSB[4 * tw + tb]
                    v3 = vt[:].rearrange("p (h x) -> p h x", x=HD + 1)
                    nc.vector.tensor_add(
                        v3[:, :, 0:HD],
                        pv[:].rearrange("p (h d) -> p h d", d=HD),
                        bv_sb[:].rearrange("p (h d) -> p h d", d=HD),
                    )
                    nc.gpsimd.tensor_copy(v3[:, :, HD], ones_sb[:, 0:NH])

                # ---- partial output projection for one window (host sums
                # the core pairs). Deferred: all four windows are emitted
                # interleaved between tq=3's attention pairs, where the PE
                # otherwise idles between exp-gated groups.
                def outproj(tqo):
                    osl = slice(512 * tqo, 512 * (tqo + 1))
                    for cc in range(NCB):
                        # window 3 alternates psum rings: attention is over,
                        # so 'ps' slots are free — 4 chains in flight instead
                        # of 2 shortens the end-of-kernel drain
                        pool_, tag_ = (psS, "ps") if tqo == 3 and cc % 2 else (psP, "pp")
                        pso = pool_.tile([P, 512], F32, tag=tag_, name="pso")
                        for fc in range(NFB):
                            o = (cc * NFB + fc) * P
                            nc.tensor.matmul(
                                pso[:], wo_sb[:, o : o + P], YA[fc][:, osl],
                                start=(fc == 0), stop=(fc == NFB - 1),
                            )
                        osb = ob_pool.tile([P, 512], BF16, tag="ob", name="osb")
                        nc.vector.tensor_scalar_add(osb[:], pso[:], bo_sb[cc])
                        nc.sync.dma_start(out[P * cc : P * (cc + 1), osl], osb[:])

                # ---- attention for query window tq == tw
                tq = tw
                ntk = 4 * (tq + 1)
                for pair in range(4):
                    psy = [
                        psY.tile([HD + 1, 512], F32, tag="psy", name=f"psy{s}")
                        for s in range(2)
                    ]
                    for g in range(2 * (tq + 1)):
                        # diagonal groups run their two key-tiles in reverse
                        # (bigger qlo first) so the fused exp window starts
                        # later — shrinks exp'd garbage. Safe because tk==0
                        # (the start=True AV) only occurs in group 0, which
                        # keeps natural order.
                        tks = [2 * g, 2 * g + 1]
                        if 2 * g - 4 * tq >= 0 and g != 0:
                            tks = [2 * g + 1, 2 * g]
                        qlos = [max(0, 128 * (tk - 4 * tq)) for tk in tks]
                        pss = [
                            psS.tile([P, 1024], F32, tag="ps", name=f"ps{s}")
                            for s in range(2)
                        ]
                        for s in range(2):
                            rows = slice(64 * s, 64 * (s + 1))
                            for u in range(2):
                                tk, qlo = tks[u], qlos[u]
                                nc.tensor.matmul(
                                    pss[s][:, 512 * u + qlo : 512 * (u + 1)],
                                    KT[pair][rows, 128 * tk : 128 * (tk + 1)],
                                    QT[pair][rows, 512 * tq + qlo : 512 * (tq + 1)],
                                    start=True, stop=True,
                                )
                        # one exp per head over both key-tiles (2 PSUM banks)
                        c0 = qlos[0]
                        exs = []
                        for s in range(2):
                            ex = ex_pool.tile([P, 1024], BF16, tag="ex", name="ex")
                            nc.scalar.activation(
                                ex[:, c0:1024], pss[s][:, c0:1024], AF.Exp,
                                scale=SCALE,
                            )
                            exs.append(ex)
                        # triangular mask on diagonal 128x128 blocks (GPSIMD)
                        for u in range(2):
                            di = tks[u] - 4 * tq
                            if di >= 0:
                                col = 512 * u + 128 * di
                                for s in range(2):
                                    nc.gpsimd.tensor_mul(
                                        exs[s][:, col : col + P],
                                        exs[s][:, col : col + P],
                                        tri_sb[:],
                                    )
                        # A @ V accumulation (+ rowsum via inline ones col)
                        for u in range(2):
                            tk, qlo = tks[u], qlos[u]
                            for s in range(2):
                                h = 2 * pair + s
                                vsl = slice((HD + 1) * h, (HD + 1) * (h + 1))
                                nc.tensor.matmul(
                                    psy[s][0 : HD + 1, qlo:512],
                                    VSB[tk][:, vsl],
                                    exs[s][:, 512 * u + qlo : 512 * (u + 1)],
                                    start=(tk == 0),
                                    stop=(g == 2 * (tq + 1) - 1 and u == 1),
                                )
                    # evacuate psy to SBUF fast (frees the PSUM bank), then
                    # normalize lazily: approx recip + DMA row-broadcast
                    yus, rbs = [], []
                    for s in range(2):
                        yu = yu_pool.tile([HD + 1, 512], F32, tag="yu", name="yu")
                        nc.vector.tensor_copy(yu[:], psy[s][0 : HD + 1, :])
                        # broadcast the raw rowsum row via DRAM bounce (SBUF
                        # APs cannot have zero partition step; DRAM APs can),
                        # THEN reciprocal on partitions 0-63 — the custom DVE
                        # recip op breaks on partition-shifted input.
                        rcd = dram.tile([1, 512], F32, tag="rcd", name="rcd")
                        nc.sync.dma_start(rcd[:], yu[HD : HD + 1, :])
                        rs = rc_pool.tile([HD, 512], F32, tag="rc", name="rs")
                        nc.sync.dma_start(rs[:], rcd[:].to_broadcast((HD, 512)))
                        rb = rb_pool.tile([HD, 512], F32, tag="rb", name="rb")
                        with nc.allow_low_precision(
                            reason="softmax recip via 18-bit approx (bf16 path)"
                        ):
                            nc.vector.reciprocal_approx_fast(
                                out=rb[:], in_=rs[:]
                            )
                        yus.append(yu)
                        rbs.append(rb)
                    for s in range(2):
                        nc.vector.tensor_mul(
                            YA[pair][64 * s : 64 * (s + 1), qsl],
                            yus[s][0:HD, :], rbs[s][:],
                        )
                    if tw == NTQ - 1:
                        outproj(pair)

    nc.compile()
    return nc


_NC_CACHE = None


def _get_nc():
    global _NC_CACHE
    if _NC_CACHE is None:
        _NC_CACHE = build_program()
    return _NC_CACHE


def _w_qk_layout(w):
    # [p, fb, cb, j] = w[128*cb + p, 128*fb + j], flattened to [P, 4096]
    return np.ascontiguousarray(
        w.reshape(NCB, P, NFB, P).transpose(1, 2, 0, 3).reshape(P, NFB * NCB * P)
    ).astype(BFNP)


def _w_o_layout(w):
    # [p, cc, fc, j] = w[128*fc + p, 128*cc + j], flattened to [P, 4096]
    return np.ascontiguousarray(
        w.reshape(NFB, P, NCB, P).transpose(1, 2, 0, 3).reshape(P, NCB * NFB * P)
    ).astype(BFNP)


def _make_in_maps(inputs) -> list:
    q = np.asarray(inputs["q"], dtype=np.float32)
    k = np.asarray(inputs["k"], dtype=np.float32)
    v = np.asarray(inputs["v"], dtype=np.float32)
    Wq = np.asarray(inputs["Wq"], dtype=np.float32)
    Wk = np.asarray(inputs["Wk"], dtype=np.float32)
    Wv = np.asarray(inputs["Wv"], dtype=np.float32)
    Wo = np.asarray(inputs["Wo"], dtype=np.float32)
    bq_f = np.asarray(inputs["bq"], dtype=np.float32)
    bk_f = np.asarray(inputs["bk"], dtype=np.float32)
    bv_f = np.asarray(inputs["bv"], dtype=np.float32)
    bo_f = np.asarray(inputs["bo"], dtype=np.float32)
    # mask is all-ones in this problem (causal handled in-kernel); ignored.

    pgrid, jgrid = np.mgrid[0:P, 0:P]
    triv = (jgrid >= pgrid).astype(BFNP)
    onesv = np.ones((P, HD), dtype=BFNP)

    # host-side transpose: x^T [C, T] per batch, shared by the core pair
    xqT = [np.ascontiguousarray(q[b].T).astype(BFNP) for b in range(4)]
    xkT = [np.ascontiguousarray(k[b].T).astype(BFNP) for b in range(4)]
    xvT = [np.ascontiguousarray(v[b].T).astype(BFNP) for b in range(4)]

    in_maps = []
    for c in range(NCORES):
        b, h2 = divmod(c, 2)
        fsl = slice(FS * h2, FS * (h2 + 1))
        in_maps.append({
            "xqt": xqT[b],
            "xkt": xkT[b],
            "xvt": xvT[b],
            "wq": _w_qk_layout(Wq[:, fsl]),
            "wk": _w_qk_layout(Wk[:, fsl]),
            "wv": np.ascontiguousarray(Wv[:, fsl]).astype(BFNP),
            "wo": _w_o_layout(Wo[fsl, :]),
            "bq": np.ascontiguousarray(bq_f[fsl].reshape(NFB, P).T),
            "bk": np.ascontiguousarray(bk_f[fsl].reshape(NFB, P).T),
            "bv": np.ascontiguousarray(
                np.broadcast_to(bv_f[fsl].reshape(1, FS), (P, FS))),
            "bo": np.ascontiguousarray((bo_f / 2.0).reshape(NCB, P).T),
            "tri": triv,
            "onesp": onesv,
        })
    return in_maps


def kernel(**inputs) -> np.ndarray:
    in_maps = _make_in_maps(inputs)
    nc = _get_nc()
    res = run_bass_kernel_spmd(nc, in_maps, list(range(NCORES)))

    full = np.empty((4, T, C), dtype=np.float32)
    for b in range(4):
        po = (res.results[2 * b]["out"].astype(np.float32)
              + res.results[2 * b + 1]["out"].astype(np.float32))
        full[b] = po.T
    return full


# revision 21
# speedup vs baseline: 1.0886x; 1.0886x over previous
"""Multi-head attention (B=4, T=2048, C=1024, H=16, causal) on 8 TRN2 cores.

Sharding: core c -> batch b = c//2, head-half h2 = c%2 (8 heads / core).
bf16 datapath (rel tol 2e-2 >> bf16 noise). Host pre-transposes x to
[C, T] so projections need no on-chip transposes. Per 512-token window:
column-parallel K/Q projections into K^T/Q^T [feat, T] layout, V natural,
then causal attention for that query window (scores 2 heads per slot via
base-64 partition row-tiling, softmax row-sums via inline ones column in
V), normalized output written to SBUF-resident y^T, row-split output
projection, host sums the core pairs.
"""

import sys

sys.path.insert(0, "/opt/trn_rl_repo")

import numpy as np
import ml_dtypes

import concourse.bacc as bacc
import concourse.bass as bass
import concourse.mybir as mybir
import concourse.tile as tile
from concourse.bass_utils import run_bass_kernel_spmd

F32 = mybir.dt.float32
BF16 = mybir.dt.bfloat16
AF = mybir.ActivationFunctionType
BFNP = ml_dtypes.bfloat16

P = 128          # partitions
T = 2048         # sequence length
C = 1024         # model dim
FS = 512         # per-core feature slice (8 heads x 64)
NH = 8           # heads per core
HD = 64          # head dim
SCALE = 0.125    # 1/sqrt(64)
NCORES = 8

NTQ = 4          # T / 512 query windows
NFB = 4          # FS / 128 feature blocks
NCB = 8          # C / 128 contraction blocks
NTT = 16         # T / 128 key tiles


def build_program():
    nc = bacc.Bacc(num_devices=NCORES)

    xqt = nc.declare_dram_parameter("xqt", [C, T], BF16, isOutput=False)
    xkt = nc.declare_dram_parameter("xkt", [C, T], BF16, isOutput=False)
    xvt = nc.declare_dram_parameter("xvt", [C, T], BF16, isOutput=False)
    # wq/wk[p, (fb*NCB+cb)*P + j] = W[128*cb + p, 512*h2 + 128*fb + j]
    wq = nc.declare_dram_parameter("wq", [P, NFB * NCB * P], BF16, isOutput=False)
    wk = nc.declare_dram_parameter("wk", [P, NFB * NCB * P], BF16, isOutput=False)
    wv = nc.declare_dram_parameter("wv", [C, FS], BF16, isOutput=False)
    # wo[p, (cc*NFB+fc)*P + j] = Wo[512*h2 + 128*fc + p, 128*cc + j]
    wo = nc.declare_dram_parameter("wo", [P, NCB * NFB * P], BF16, isOutput=False)
    bq = nc.declare_dram_parameter("bq", [P, NFB], F32, isOutput=False)
    bk = nc.declare_dram_parameter("bk", [P, NFB], F32, isOutput=False)
    bv = nc.declare_dram_parameter("bv", [P, FS], F32, isOutput=False)
    bo = nc.declare_dram_parameter("bo", [P, NCB], F32, isOutput=False)
    # tri[p, j] = 1.0 iff j >= p  (causal mask for a 128x128 diagonal block)
    tri = nc.declare_dram_parameter("tri", [P, P], BF16, isOutput=False)
    onesp = nc.declare_dram_parameter("onesp", [P, HD], BF16, isOutput=False)
    out = nc.declare_dram_parameter("out", [C, T], BF16, isOutput=True)

    with tile.TileContext(nc) as tc:
        import contextlib

        with contextlib.ExitStack() as ctx:
            consts = ctx.enter_context(tc.tile_pool(name="consts", bufs=1))
            xw_pool = ctx.enter_context(tc.tile_pool(name="xw", bufs=2))
            kt_pool = ctx.enter_context(tc.tile_pool(name="ktp", bufs=1))
            qt_pool = ctx.enter_context(tc.tile_pool(name="qtp", bufs=1))
            v_pool = ctx.enter_context(tc.tile_pool(name="vp", bufs=1))
            ya_pool = ctx.enter_context(tc.tile_pool(name="yap", bufs=1))
            ex_pool = ctx.enter_context(tc.tile_pool(name="expp", bufs=4))
            yu_pool = ctx.enter_context(tc.tile_pool(name="yup", bufs=4))
            rc_pool = ctx.enter_context(tc.tile_pool(name="rcp", bufs=4))
            rb_pool = ctx.enter_context(tc.tile_pool(name="rbp", bufs=4))
            ob_pool = ctx.enter_context(tc.tile_pool(name="obp", bufs=3))
            # PSUM: 'ps' [P,1024]x2 = 4 banks, 'psy' x2 = 2, 'pp' x2 = 2
            psS = ctx.enter_context(tc.tile_pool(name="psS", bufs=2, space="PSUM"))
            psY = ctx.enter_context(tc.tile_pool(name="psY", bufs=2, space="PSUM"))
            psP = ctx.enter_context(tc.tile_pool(name="psP", bufs=2, space="PSUM"))
            dram = ctx.enter_context(tc.tile_pool(name="dram", bufs=4, space="DRAM"))

            # ---- x^T window loader (one DMA per matrix, [c, cb-major, t])
            def load_xwin(tw_):
                sl = slice(512 * tw_, 512 * (tw_ + 1))
                xk_w = xw_pool.tile([P, NCB, 512], BF16, tag="xk", name="xk_w")
                nc.sync.dma_start(
                    xk_w[:], xkt[:, sl].rearrange("(cb p) t -> p cb t", p=P)
                )
                xq_w = xw_pool.tile([P, NCB, 512], BF16, tag="xq", name="xq_w")
                nc.sync.dma_start(
                    xq_w[:], xqt[:, sl].rearrange("(cb p) t -> p cb t", p=P)
                )
                xv_w = xw_pool.tile([P, NCB, 512], BF16, tag="xv", name="xv_w")
                nc.sync.dma_start(
                    xv_w[:], xvt[:, sl].rearrange("(cb p) t -> p cb t", p=P)
                )
                return xk_w, xq_w, xv_w

            # ---- constants, ordered so the first K-projection chain's
            # operands (wk, xk window 0) land first on the queue
            wk_sb = consts.tile([P, NFB * NCB * P], BF16, tag="wk", name="wk_sb")
            nc.sync.dma_start(wk_sb[:], wk[:])
            bk_t = consts.tile([P, NFB], F32, tag="bk", name="bk_t")
            nc.sync.dma_start(bk_t[:], bk[:])
            xwin0 = load_xwin(0)
            wq_sb = consts.tile([P, NFB * NCB * P], BF16, tag="wq", name="wq_sb")
            nc.sync.dma_start(wq_sb[:], wq[:])
            bq_t = consts.tile([P, NFB], F32, tag="bq", name="bq_t")
            nc.sync.dma_start(bq_t[:], bq[:])
            wv_sb = consts.tile([P, NCB, FS], BF16, tag="wv", name="wv_sb")
            nc.sync.dma_start(
                wv_sb[:], wv[:].rearrange("(cb p) f -> p cb f", p=P)
            )
            bv_sb = consts.tile([P, FS], F32, tag="bv", name="bv_sb")
            nc.sync.dma_start(bv_sb[:], bv[:])
            ones_sb = consts.tile([P, HD], BF16, tag="ones", name="ones_sb")
            nc.sync.dma_start(ones_sb[:], onesp[:])
            tri_sb = consts.tile([P, P], BF16, tag="tri", name="tri_sb")
            nc.sync.dma_start(tri_sb[:], tri[:])
            wo_sb = consts.tile([P, NCB * NFB * P], BF16, tag="wo", name="wo_sb")
            nc.sync.dma_start(wo_sb[:], wo[:])
            bo_t = consts.tile([P, NCB], F32, tag="bo", name="bo_t")
            nc.sync.dma_start(bo_t[:], bo[:])
            bq_sb = [bq_t[:, i : i + 1] for i in range(NFB)]
            bk_sb = [bk_t[:, i : i + 1] for i in range(NFB)]
            bo_sb = [bo_t[:, i : i + 1] for i in range(NCB)]

            # ---- persistent attention operands (bf16)
            KT = [kt_pool.tile([P, T], BF16, tag=f"kt{i}", name=f"kt{i}")
                  for i in range(NFB)]
            QT = [qt_pool.tile([P, T], BF16, tag=f"qt{i}", name=f"qt{i}")
                  for i in range(NFB)]
            # V tiles carry an inline ones column per head: [v_h | 1] x 8
            VSB = [v_pool.tile([P, NH * (HD + 1)], BF16, tag=f"v{i}", name=f"v{i}")
                   for i in range(NTT)]
            YA = [ya_pool.tile([P, T], BF16, tag=f"ya{i}", name=f"ya{i}")
                  for i in range(NFB)]

            for tw in range(NTQ):
                qsl = slice(512 * tw, 512 * (tw + 1))

                xk_w, xq_w, xv_w = xwin0 if tw == 0 else load_xwin(tw)

                # ---- K^T, Q^T projections for this window
                for wsb, xin, bias_sb, OUT in (
                    (wk_sb, xk_w, bk_sb, KT),
                    (wq_sb, xq_w, bq_sb, QT),
                ):
                    for fb in range(NFB):
                        pp = psP.tile([P, 512], F32, tag="pp", name="pp")
                        for cb in range(NCB):
                            o = (fb * NCB + cb) * P
                            nc.tensor.matmul(
                                pp[:], wsb[:, o : o + P], xin[:, cb, :],
                                start=(cb == 0), stop=(cb == NCB - 1),
                            )
                        nc.vector.tensor_scalar_add(
                            OUT[fb][:, qsl], pp[:], bias_sb[fb]
                        )

                # ---- V projection (natural layout) for this window
                for tb in range(4):
                    pv = psP.tile([P, 512], F32, tag="pp", name="pv")
                    for cb in range(NCB):
                        nc.tensor.matmul(
                            pv[:], xv_w[:, cb, 128 * tb : 128 * (tb + 1)],
                            wv_sb[:, cb, :],
                            start=(cb == 0), stop=(cb == NCB - 1),
                        )
                    vt = VSB[4 * tw + tb]
                    v3 = vt[:].rearrange("p (h x) -> p h x", x=HD + 1)
                    nc.vector.tensor_add(
                        v3[:, :, 0:HD],
                        pv[:].rearrange("p (h d) -> p h d", d=HD),
                        bv_sb[:].rearrange("p (h d) -> p h d", d=HD),
                    )
                    nc.gpsimd.tensor_copy(v3[:, :, HD], ones_sb[:, 0:NH])

                # ---- partial output projection for one window (host sums
                # the core pairs). Deferred: all four windows are emitted
                # interleaved between tq=3's attention pairs, where the PE
                # otherwise idles between exp-gated groups.
                def outproj(tqo):
                    osl = slice(512 * tqo, 512 * (tqo + 1))
                    for cc in range(NCB):
                        # window 3 alternates psum rings: attention is over,
                        # so 'ps' slots are free — 4 chains in flight instead
                        # of 2 shortens the end-of-kernel drain
                        pool_, tag_ = (psS, "ps") if tqo == 3 and cc % 2 else (psP, "pp")
                        pso = pool_.tile([P, 512], F32, tag=tag_, name="pso")
                        for fc in range(NFB):
                            o = (cc * NFB + fc) * P
                            nc.tensor.matmul(
                                pso[:], wo_sb[:, o : o + P], YA[fc][:, osl],
                                start=(fc == 0), stop=(fc == NFB - 1),
                            )
                        osb = ob_pool.tile([P, 512], BF16, tag="ob", name="osb")
                        nc.vector.tensor_scalar_add(osb[:], pso[:], bo_sb[cc])
                        nc.sync.dma_start(out[P * cc : P * (cc + 1), osl], osb[:])

                # ---- attention for query window tq == tw
                tq = tw
                ntk = 4 * (tq + 1)
                for pair in range(4):
                    psy = [
                        psY.tile([HD + 1, 512], F32, tag="psy", name=f"psy{s}")
                        for s in range(2)
                    ]
                    for g in range(2 * (tq + 1)):
                        # diagonal groups run their two key-tiles in reverse
                        # (bigger qlo first) so the fused exp window starts
                        # later — shrinks exp'd garbage. Safe because tk==0
                        # (the start=True AV) only occurs in group 0, which
                        # keeps natural order.
                        tks = [2 * g, 2 * g + 1]
                        if 2 * g - 4 * tq >= 0 and g != 0:
                            tks = [2 * g + 1, 2 * g]
                        qlos = [max(0, 128 * (tk - 4 * tq)) for tk in tks]
                        pss = [
                            psS.tile([P, 1024], F32, tag="ps", name=f"ps{s}")
                            for s in range(2)
                        ]
                        for s in range(2):
                            rows = slice(64 * s, 64 * (s + 1))
                            for u in range(2):
                                tk, qlo = tks[u], qlos[u]
                                nc.tensor.matmul(
                                    pss[s][:, 512 * u + qlo : 512 * (u + 1)],
                                    KT[pair][rows, 128 * tk : 128 * (tk + 1)],
                                    QT[pair][rows, 512 * tq + qlo : 512 * (tq + 1)],
                                    start=True, stop=True,
                                )
                        # one exp per head over both key-tiles (2 PSUM banks)
                        c0 = qlos[0]
                        exs = []
                        for s in range(2):
                            ex = ex_pool.tile([P, 1024], BF16, tag="ex", name="ex")
                            nc.scalar.activation(
                                ex[:, c0:1024], pss[s][:, c0:1024], AF.Exp,
                                scale=SCALE,
                            )
                            exs.append(ex)
                        # triangular mask on diagonal 128x128 blocks (GPSIMD)
                        for u in range(2):
                            di = tks[u] - 4 * tq
                            if di >= 0:
                                col = 512 * u + 128 * di
                                for s in range(2):
                                    nc.gpsimd.tensor_mul(
                                        exs[s][:, col : col + P],
                                        exs[s][:, col : col + P],
                                        tri_sb[:],
                                    )
                        # A @ V accumulation (+ rowsum via inline ones col)
                        for u in range(2):
                            tk, qlo = tks[u], qlos[u]
                            for s in range(2):
                                h = 2 * pair + s
                                vsl = slice((HD + 1) * h, (HD + 1) * (h + 1))
                                nc.tensor.matmul(
                                    psy[s][0 : HD + 1, qlo:512],
                                    VSB[tk][:, vsl],
                                    exs[s][:, 512 * u + qlo : 512 * (u + 1)],
                                    start=(tk == 0),
                                    stop=(g == 2 * (tq + 1) - 1 and u == 1),
                                )
                    # evacuate psy to SBUF fast (frees the PSUM bank), then
                    # normalize lazily: approx recip + DMA row-broadcast
                    yus, rbs = [], []
                    for s in range(2):
                        yu = yu_pool.tile([HD + 1, 512], F32, tag="yu", name="yu")
                        nc.vector.tensor_copy(yu[:], psy[s][0 : HD + 1, :])
                        # broadcast the raw rowsum row via DRAM bounce (SBUF
                        # APs cannot have zero partition step; DRAM APs can),
                        # THEN reciprocal on partitions 0-63 — the custom DVE
                        # recip op breaks on partition-shifted input.
                        rcd = dram.tile([1, 512], F32, tag="rcd", name="rcd")
                        nc.sync.dma_start(rcd[:], yu[HD : HD + 1, :])
                        rs = rc_pool.tile([HD, 512], F32, tag="rc", name="rs")
                        nc.sync.dma_start(rs[:], rcd[:].to_broadcast((HD, 512)))
                        rb = rb_pool.tile([HD, 512], F32, tag="rb", name="rb")
                        with nc.allow_low_precision(
                            reason="softmax recip via 18-bit approx (bf16 path)"
                        ):
                            nc.vector.reciprocal_approx_fast(
                                out=rb[:], in_=rs[:]
                            )
                        yus.append(yu)
                        rbs.append(rb)
                    for s in range(2):
                        nc.vector.tensor_mul(
                            YA[pair][64 * s : 64 * (s + 1), qsl],
                            yus[s][0:HD, :], rbs[s][:],
                        )
                    if tw == NTQ - 1:
                        outproj(pair)

    nc.compile()
    return nc


_NC_CACHE = None


def _get_nc():
    global _NC_CACHE
    if _NC_CACHE is None:
        _NC_CACHE = build_program()
    return _NC_CACHE


def _w_qk_layout(w):
    # [p, fb, cb, j] = w[128*cb + p, 128*fb + j], flattened to [P, 4096]
    return np.ascontiguousarray(
        w.reshape(NCB, P, NFB, P).transpose(1, 2, 0, 3).reshape(P, NFB * NCB * P)
    ).astype(BFNP)


def _w_o_layout(w):
    # [p, cc, fc, j] = w[128*fc + p, 128*cc + j], flattened to [P, 4096]
    return np.ascontiguousarray(
        w.reshape(NFB, P, NCB, P).transpose(1, 2, 0, 3).reshape(P, NCB * NFB * P)
    ).astype(BFNP)


def _make_in_maps(inputs) -> list:
    q = np.asarray(inputs["q"], dtype=np.float32)
    k = np.asarray(inputs["k"], dtype=np.float32)
    v = np.asarray(inputs["v"], dtype=np.float32)
    Wq = np.asarray(inputs["Wq"], dtype=np.float32)
    Wk = np.asarray(inputs["Wk"], dtype=np.float32)
    Wv = np.asarray(inputs["Wv"], dtype=np.float32)
    Wo = np.asarray(inputs["Wo"], dtype=np.float32)
    bq_f = np.asarray(inputs["bq"], dtype=np.float32)
    bk_f = np.asarray(inputs["bk"], dtype=np.float32)
    bv_f = np.asarray(inputs["bv"], dtype=np.float32)
    bo_f = np.asarray(inputs["bo"], dtype=np.float32)
    # mask is all-ones in this problem (causal handled in-kernel); ignored.

    pgrid, jgrid = np.mgrid[0:P, 0:P]
    triv = (jgrid >= pgrid).astype(BFNP)
    onesv = np.ones((P, HD), dtype=BFNP)

    # host-side transpose: x^T [C, T] per batch, shared by the core pair
    xqT = [np.ascontiguousarray(q[b].T).astype(BFNP) for b in range(4)]
    xkT = [np.ascontiguousarray(k[b].T).astype(BFNP) for b in range(4)]
    xvT = [np.ascontiguousarray(v[b].T).astype(BFNP) for b in range(4)]

    in_maps = []
    for c in range(NCORES):
        b, h2 = divmod(c, 2)
        fsl = slice(FS * h2, FS * (h2 + 1))
        in_maps.append({
            "xqt": xqT[b],
            "xkt": xkT[b],
            "xvt": xvT[b],
            "wq": _w_qk_layout(Wq[:, fsl]),
            "wk": _w_qk_layout(Wk[:, fsl]),
            "wv": np.ascontiguousarray(Wv[:, fsl]).astype(BFNP),
            "wo": _w_o_layout(Wo[fsl, :]),
            "bq": np.ascontiguousarray(bq_f[fsl].reshape(NFB, P).T),
            "bk": np.ascontiguousarray(bk_f[fsl].reshape(NFB, P).T),
            "bv": np.ascontiguousarray(
                np.broadcast_to(bv_f[fsl].reshape(1, FS), (P, FS))),
            "bo": np.ascontiguousarray((bo_f / 2.0).reshape(NCB, P).T),
            "tri": triv,
            "onesp": onesv,
        })
    return in_maps


def kernel(**inputs) -> np.ndarray:
    in_maps = _make_in_maps(inputs)
    nc = _get_nc()
    res = run_bass_kernel_spmd(nc, in_maps, list(range(NCORES)))

    full = np.empty((4, T, C), dtype=np.float32)
    for b in range(4):
        po = (res.results[2 * b]["out"].astype(np.float32)
              + res.results[2 * b + 1]["out"].astype(np.float32))
        full[b] = po.T
    return full
